# revision 1
# baseline (speedup 1.0000x reference)
"""Trainium2 Bass kernel for nn_DINOBevAligner (BEVFormer-style view aligner).

Strategy (8 NeuronCores, query-sector sharded, ZERO cross-core comm):
  - The 2500 BEV queries are sorted by azimuth and split into 8 sectors of
    320 (padded).  Each core receives, for every camera view, only the image
    COLUMNS its sector's queries bilinearly sample (contiguous x-ranges,
    host-computed from lidar2img), with all 768 channels, in bf16.
  - With full channels local, pre-LN (per token), the masked bilinear
    gather + view-weighted reduction, post-LN (per query) and the grouped
    softmax reducer are all core-local: no collectives at all.
  - The gather is dense TensorEngine matmuls: tokens stored x-major
    (n = x*37 + y) in 128-token tiles (view-aligned); the host builds one
    dense (128 x 320) bf16 weight block per tile (bilinear weights * pillar
    mask).  LayerNorm folds in as: W' = W * rsqrt(var+eps) * softplus(w_view)
    (per-token row scale on device) and a rank-1 mean correction row K(q)
    accumulated by 1-column matmuls and subtracted during the PSUM->SBUF copy.
  - The grouped reducer (C=768 -> 256 with softmax(logits) weights and
    post-LN gamma/beta) is 12 more (128x128)x(128x320) matmuls.
Host work: projection / index / weight-block construction (sampling-operator
descriptors derived from the 6 4x4 matrices) and input/output relayout.
All tensor math runs on device.
"""
import sys

sys.path.insert(0, "/opt/trn_rl_repo")

import numpy as np
import ml_dtypes

BEV_H, BEV_W = 50, 50
D_PILLAR = 4
PC = (-51.2, -51.2, -5.0, 51.2, 51.2, 3.0)
S_IMG = 518.0
LN_EPS = 1e-5
FUSE_EPS = 1e-6
C_CTX = 256
Q = BEV_H * BEV_W
QP = 2560
NCORE = 8
SEC = QP // NCORE            # 320 queries per core
TOK_TILE = 128
V = 6
C = 768
NCH = C // 128               # 6 channel chunks of 128
NKH = C_CTX // 128           # 2 output-channel halves


# ----------------------------------------------------------------- host math
def _projection_np(lidar2img):
    dt = np.float32
    Z = int(round(PC[5] - PC[2]))
    zs = (np.linspace(0.5, Z - 0.5, D_PILLAR, dtype=dt) / dt(Z))[:, None, None]
    xs = (np.linspace(0.5, BEV_W - 0.5, BEV_W, dtype=dt) / dt(BEV_W))[None, None, :]
    ys = (np.linspace(0.5, BEV_H - 0.5, BEV_H, dtype=dt) / dt(BEV_H))[None, :, None]
    x, y, z = np.broadcast_arrays(xs, ys, zs)
    ref = np.stack([x, y, z], axis=-1).reshape(D_PILLAR, Q, 3).astype(dt)
    ref = ref * np.array([PC[3] - PC[0], PC[4] - PC[1], PC[5] - PC[2]], dt) \
        + np.array([PC[0], PC[1], PC[2]], dt)
    ref4 = np.concatenate([ref, np.ones_like(ref[..., :1])], axis=-1)
    pts = np.einsum('bvij,dqj->bdvqi', lidar2img.astype(dt), ref4)
    zc = pts[..., 2]
    valid = zc > 1e-5
    uv = pts[..., :2] / np.maximum(zc, dt(1e-5))[..., None] / dt(S_IMG)
    u, v = uv[..., 0], uv[..., 1]
    valid = valid & (u > 0.0) & (u < 1.0) & (v > 0.0) & (v < 1.0)
    tr = lambda a: np.transpose(a, (0, 2, 3, 1))
    return tr(u), tr(v), tr(valid)


def build_plan(lidar2img, patch_h, patch_w):
    """Sector plan: per core, the referenced x-column ranges per view, the
    view-aligned local token tiling, dense per-tile weight blocks, counts."""
    dt = np.float32
    Hp, Wp = int(patch_h), int(patch_w)
    u, v, valid = _projection_np(lidar2img)
    u, v, valid = u[0], v[0], valid[0]              # (V,Q,D)

    x_p = (u * dt(S_IMG) + dt(0.5)) / dt(S_IMG) * dt(Wp) - dt(0.5)
    y_p = (v * dt(S_IMG) + dt(0.5)) / dt(S_IMG) * dt(Hp) - dt(0.5)
    x0 = np.floor(x_p); fx = x_p - x0; x0 = x0.astype(np.int64)
    y0 = np.floor(y_p); fy = y_p - y0; y0 = y0.astype(np.int64)
    m = valid.astype(dt)
    cnt = m.sum(axis=-1)                            # (V,Q)

    toks = np.full((V, Q, D_PILLAR, 4), -1, dtype=np.int64)
    wts = np.zeros((V, Q, D_PILLAR, 4), dtype=dt)
    ci = 0
    for dx in (0, 1):
        for dy in (0, 1):
            xi, yi = x0 + dx, y0 + dy
            inb = (xi >= 0) & (xi < Wp) & (yi >= 0) & (yi < Hp)
            w = np.where(dx, fx, 1 - fx) * np.where(dy, fy, 1 - fy) * inb.astype(dt)
            w = w * m
            n_xmaj = np.clip(xi, 0, Wp - 1) * Hp + np.clip(yi, 0, Hp - 1)
            live = (w != 0) & inb
            toks[..., ci] = np.where(live, n_xmaj, -1)
            wts[..., ci] = np.where(live, w, 0)
            ci += 1
    tk = toks.reshape(V, Q, 16)
    wt = wts.reshape(V, Q, 16)

    qy, qx = np.divmod(np.arange(Q), BEV_W)
    az = np.arctan2(qy - (BEV_H - 1) / 2.0, qx - (BEV_W - 1) / 2.0)
    perm = np.argsort(az, kind='stable').astype(np.int64)   # position -> orig q

    cores = []
    for k in range(NCORE):
        qs = perm[k * SEC:min((k + 1) * SEC, Q)]    # orig q at local col j
        views = []                                  # (v, xlo, ntok, base_tile)
        base = 0
        for vv in range(V):
            msk = wt[vv][qs] != 0                   # (nq,16)
            if not msk.any():
                continue
            cols = tk[vv][qs][msk] // Hp
            xlo, xhi = int(cols.min()), int(cols.max())
            ntok = (xhi - xlo + 1) * Hp
            ntile = (ntok + TOK_TILE - 1) // TOK_TILE
            views.append((vv, xlo, ntok, base))
            base += ntile
        cores.append(dict(qs=qs, views=views, ntil=base))
    NTIL = max(c["ntil"] for c in cores)

    for k, ck in enumerate(cores):
        qs = ck["qs"]
        nq = len(qs)
        Wb = np.zeros((NTIL, TOK_TILE, SEC), dtype=dt)
        vmap = np.zeros((NTIL, V), dtype=dt)
        for (vv, xlo, ntok, base) in ck["views"]:
            ntile = (ntok + TOK_TILE - 1) // TOK_TILE
            vmap[base:base + ntile, vv] = 1.0
            tkv = tk[vv][qs]                        # (nq, 16)
            wtv = wt[vv][qs]
            rows, cols16 = np.nonzero(wtv)
            for r, c16 in zip(rows, cols16):
                n = tkv[r, c16]
                l = (n // Hp - xlo) * Hp + (n % Hp)
                Wb[base + l // TOK_TILE, l % TOK_TILE, r] += wtv[r, c16]
        cntq = np.zeros((SEC, V), dtype=dt)
        cntq[:nq] = cnt.T[qs]
        ck["wmat"] = Wb
        ck["vmap"] = vmap
        ck["cntq"] = cntq
    return dict(perm=perm, cores=cores, NTIL=NTIL, Hp=Hp, Wp=Wp)


def retile_tokens_sector(last_tokens, plan):
    """Per-core (128, NTIL, 768) bf16 token arrays (x-major, view-aligned)."""
    B, Vv, N, Cc = last_tokens.shape
    Hp, Wp = plan["Hp"], plan["Wp"]
    NTIL = plan["NTIL"]
    # x-major f32 view of tokens once
    xm = np.transpose(last_tokens[0].reshape(Vv, Hp, Wp, Cc),
                      (0, 2, 1, 3)).reshape(Vv, Wp * Hp, Cc)
    outs = []
    for ck in plan["cores"]:
        arr = np.zeros((NTIL * TOK_TILE, Cc), dtype=np.float32)
        for (vv, xlo, ntok, base) in ck["views"]:
            seg = xm[vv, xlo * Hp:xlo * Hp + ntok]
            arr[base * TOK_TILE:base * TOK_TILE + ntok] = seg
        a = arr.reshape(NTIL, TOK_TILE, Cc).transpose(1, 0, 2)
        outs.append(np.ascontiguousarray(a.astype(ml_dtypes.bfloat16)))
    return outs


# -------------------------------------------------------------- bass program
def build_program(NTIL):
    import concourse.bass as bass
    import concourse.bacc as bacc
    import concourse.tile as tile
    from concourse import mybir

    f32 = mybir.dt.float32
    bf16 = mybir.dt.bfloat16
    AF = mybir.ActivationFunctionType
    ALU = mybir.AluOpType

    nc = bacc.Bacc("TRN2", target_bir_lowering=False, debug=False,
                   num_devices=NCORE)

    tok_d = nc.dram_tensor("tok", [128, NTIL * C], bf16, kind="ExternalInput")
    wmat_d = nc.dram_tensor("wmat", [128, NTIL * SEC], bf16,
                            kind="ExternalInput")
    cnt_d = nc.dram_tensor("cnt", [1, SEC * V], f32, kind="ExternalInput")
    rowc_d = nc.dram_tensor("rowc", [1, 2 * C + C + 8], f32,
                            kind="ExternalInput")   # gamma | beta | logits | wv
    m1m_d = nc.dram_tensor("m1mask", [128, NCH * NKH * 128], bf16,
                           kind="ExternalInput")
    vmap_d = nc.dram_tensor("vmap", [128, NTIL * V], f32, kind="ExternalInput")
    ones_d = nc.dram_tensor("onescol", [128, 8], bf16, kind="ExternalInput")
    out_d = nc.dram_tensor("out", [128, NKH * SEC], f32, kind="ExternalOutput")

    GRP = []
    t0 = 0
    for gs in (1, 1, 2, 2, 3, 3, 3, 3, 3):
        if t0 >= NTIL:
            break
        GRP.append((t0, min(t0 + gs, NTIL)))
        t0 = GRP[-1][1]
    while t0 < NTIL:
        GRP.append((t0, min(t0 + 3, NTIL)))
        t0 += 3

    with tile.TileContext(nc) as tc:
        with (
            tc.tile_pool(name="big", bufs=1) as big,
            tc.tile_pool(name="small", bufs=1) as small,
            tc.tile_pool(name="psum", bufs=1, space="PSUM") as psum,
            tc.tile_pool(name="dram", bufs=1, space="DRAM") as dram,
        ):
            # pre-load the one ACT table covering Exp/Ln/Square/Copy/Identity
            # (natural_log_exp_and_others, id 6): zero table reloads.
            nc.scalar.add_instruction(mybir.InstLoadActFuncSet(
                name=f"I-{nc.next_id()}", act_func_set_id=6, ins=[], outs=[]))

            # ---- tiny control DMAs first (Sync queue head)
            rowS = small.tile([1, 2 * C + C + 8], f32, tag="rowS")
            nc.sync.dma_start(out=rowS[:], in_=rowc_d.ap())
            vmapS = small.tile([128, NTIL, V], f32, tag="vmapS")
            nc.sync.dma_start(out=vmapS[:],
                              in_=vmap_d.ap().rearrange("p (t v) -> p t v", v=V))

            gam = rowS[0:1, 0:C]
            bet = rowS[0:1, C:2 * C]
            lgt = rowS[0:1, 2 * C:3 * C]
            wvr = rowS[0:1, 3 * C:3 * C + V]

            # ---- softplus(w_view) + cross-partition broadcast, ASAP
            wvp = small.tile([1, V], f32, tag="wvp")
            nc.scalar.activation(out=wvp[:], in_=wvr, func=AF.Exp)
            nc.vector.tensor_scalar_add(wvp[:], wvp[:], 1.0)
            nc.scalar.activation(out=wvp[:], in_=wvp[:], func=AF.Ln)
            smallrt = dram.tile([1, 1056], f32, tag="smallrt")
            # broadcast softplus(wv) across partitions via an idle-PE
            # ones-matmul (same proven shape as the abP broadcast: bf16,
            # K=1, M=128, N=320), bypassing the busy Sync DMA queue.
            onesrowE = small.tile([1, 128], bf16, tag="onesrowE")
            nc.vector.memset(onesrowE[:], 1.0)
            wvpb = small.tile([1, 512], bf16, tag="wvpb")
            nc.vector.memset(wvpb[:], 0.0)
            nc.vector.tensor_copy(out=wvpb[0:1, 0:V], in_=wvp[:])
            wvpr = small.tile([1, V], f32, tag="wvpr")
            nc.vector.tensor_copy(out=wvpr[:], in_=wvpb[0:1, 0:V])
            wvbP = psum.tile([128, 512], f32, tag="kbp")
            nc.tensor.matmul(wvbP[:, 0:320], lhsT=onesrowE[:],
                             rhs=wvpb[0:1, 0:320],
                             start=True, stop=True, skip_group_check=True)
            wvb = small.tile([128, V], f32, tag="wvb")
            nc.scalar.copy(out=wvb[:], in_=wvbP[:, 0:V])

            # ---- bulk input DMAs, per tile group
            tok_v = tok_d.ap().rearrange("p (t c) -> p t c", c=C)
            w_v = wmat_d.ap().rearrange("p (t q) -> p t q", q=SEC)
            tokG, wG = [], []
            for gi, (g0, g1) in enumerate(GRP):
                tg = big.tile([128, g1 - g0, C], bf16, tag=f"tok{gi}",
                              name=f"tok{gi}")
                wg_ = big.tile([128, g1 - g0, SEC], bf16, tag=f"w{gi}",
                               name=f"w{gi}")
                nc.sync.dma_start(out=tg[:], in_=tok_v[:, g0:g1, :])
                nc.sync.dma_start(out=wg_[:], in_=w_v[:, g0:g1, :])
                tokG.append(tg)
                wG.append(wg_)
            cntS = small.tile([1, SEC, V], f32, tag="cntS")
            nc.sync.dma_start(out=cntS[:],
                              in_=cnt_d.ap().rearrange("o (q v) -> o q v", v=V))
            m1S = small.tile([128, NCH, NKH * 128], bf16, tag="m1S")
            nc.sync.dma_start(
                out=m1S[:],
                in_=m1m_d.ap().rearrange("p (c j) -> p c j", c=NCH))
            onesS = small.tile([128, 8], bf16, tag="onesS")
            nc.sync.dma_start(out=onesS[:], in_=ones_d.ap())

            # ---------------- per-group: stats -> W row scale -> matmuls
            accP = psum.tile([128, NCH, 512], f32, tag="accp")
            miscP = psum.tile([128, 512], f32, tag="miscp")
            sqscr = small.tile([128, C], bf16, tag="sqscr")
            for gi, (g0, g1) in enumerate(GRP):
                gn = g1 - g0
                tg, wg_ = tokG[gi], wG[gi]
                mu = small.tile([128, gn], bf16, tag=f"mu{gi}", name=f"mu{gi}")
                varT = small.tile([128, gn], f32, tag=f"varT{gi}",
                                  name=f"varT{gi}")
                tA = small.tile([128, gn], f32, tag=f"tA{gi}", name=f"tA{gi}")
                ssq = small.tile([128, gn], f32, tag=f"ssq{gi}", name=f"ssq{gi}")
                sT = small.tile([128, gn], f32, tag=f"sT{gi}", name=f"sT{gi}")
                wvt = small.tile([128, gn, V], f32, tag=f"wvt{gi}",
                                 name=f"wvt{gi}")
                inv = small.tile([128, gn], f32, tag=f"inv{gi}",
                                 name=f"inv{gi}")
                with nc.allow_low_precision(reason="bf16 token mean: 2x DVE"):
                    nc.vector.tensor_reduce(out=mu[:], in_=tg[:],
                                            axis=mybir.AxisListType.X,
                                            op=ALU.add)
                    nc.vector.tensor_scalar_mul(mu[:], mu[:], 1.0 / C)
                for t in range(gn):
                    nc.scalar.activation(out=sqscr[:], in_=tg[:, t, :],
                                         func=AF.Square,
                                         accum_out=ssq[:, t:t + 1])
                nc.vector.tensor_tensor(out=tA[:], in0=mu[:], in1=mu[:],
                                        op=ALU.mult)
                nc.vector.tensor_scalar(out=varT[:], in0=ssq[:],
                                        scalar1=1.0 / C, scalar2=LN_EPS,
                                        op0=ALU.mult, op1=ALU.add)
                nc.vector.tensor_tensor(out=varT[:], in0=varT[:], in1=tA[:],
                                        op=ALU.subtract)
                # inv = rsqrt(var+eps) = exp(-0.5*ln(var+eps))
                nc.scalar.activation(out=inv[:], in_=varT[:], func=AF.Ln)
                nc.scalar.activation(out=inv[:], in_=inv[:], func=AF.Exp,
                                     scale=-0.5)
                # s = inv * softplus(wv[view(tile)])
                nc.vector.tensor_tensor(out=wvt[:], in0=vmapS[:, g0:g1, :],
                                        in1=wvb[:].unsqueeze(1)
                                        .broadcast_to([128, gn, V]),
                                        op=ALU.mult)
                nc.vector.tensor_reduce(out=sT[:], in_=wvt[:],
                                        axis=mybir.AxisListType.X, op=ALU.add)
                nc.vector.tensor_tensor(out=sT[:], in0=sT[:], in1=inv[:],
                                        op=ALU.mult)
                if gi % 2 == 0:
                    for t in range(gn):
                        nc.scalar.activation(out=wg_[:, t, :], in_=wg_[:, t, :],
                                             func=AF.Copy,
                                             scale=sT[:, t:t + 1])
                else:
                    with nc.allow_low_precision(reason="bf16 W row scale"):
                        for t in range(gn):
                            nc.vector.tensor_scalar_mul(wg_[:, t, :],
                                                        wg_[:, t, :],
                                                        sT[:, t:t + 1])
                for t in range(gn):
                    tglob = g0 + t
                    for ci in range(NCH):
                        nc.tensor.matmul(accP[:, ci, 0:SEC],
                                         lhsT=tg[:, t, 128 * ci:128 * (ci + 1)],
                                         rhs=wg_[:, t, :],
                                         start=(tglob == 0),
                                         stop=(tglob == NTIL - 1),
                                         skip_group_check=True)

            # ---------------- epilogue prep (overlaps the matmul phase)
            onesrow = small.tile([1, 128], bf16, tag="onesrow")
            nc.vector.memset(onesrow[:], 1.0)
            eL = small.tile([1, C], f32, tag="eL")
            nc.scalar.activation(out=eL[:], in_=lgt, func=AF.Exp)
            sL = small.tile([1, C_CTX], f32, tag="sL")
            nc.vector.tensor_reduce(out=sL[:],
                                    in_=eL[:].rearrange("o (k g) -> o k g", g=3),
                                    axis=mybir.AxisListType.X, op=ALU.add)
            nc.scalar.activation(out=sL[:], in_=sL[:], func=AF.Ln)
            wgf = small.tile([1, C], f32, tag="wgf")
            nc.vector.tensor_tensor(
                out=wgf[:].rearrange("o (k g) -> o k g", g=3),
                in0=lgt.rearrange("o (k g) -> o k g", g=3),
                in1=sL[:].unsqueeze(2).broadcast_to([1, C_CTX, 3]),
                op=ALU.subtract)
            nc.scalar.activation(out=wgf[:], in_=wgf[:], func=AF.Exp)
            valsr = small.tile([1, C], f32, tag="valsr")
            nc.vector.tensor_tensor(out=valsr[:], in0=wgf[:], in1=gam, op=ALU.mult)
            g2t = small.tile([1, C], f32, tag="g2t")
            nc.vector.tensor_tensor(out=g2t[:], in0=wgf[:], in1=bet, op=ALU.mult)
            g2r = small.tile([1, C_CTX], f32, tag="g2r")
            nc.vector.tensor_reduce(out=g2r[:],
                                    in_=g2t[:].rearrange("o (k g) -> o k g", g=3),
                                    axis=mybir.AxisListType.X, op=ALU.add)
            ng1 = small.tile([1, C_CTX], f32, tag="ng1")
            nc.vector.tensor_reduce(out=ng1[:],
                                    in_=valsr[:].rearrange("o (k g) -> o k g", g=3),
                                    axis=mybir.AxisListType.X, op=ALU.add,
                                    negate=True)
            ng1b = small.tile([1, C_CTX], bf16, tag="ng1b")
            nc.vector.tensor_copy(out=ng1b[:], in_=ng1[:])
            nc.sync.dma_start(out=smallrt[0:1, 0:C], in_=valsr[:])
            nc.sync.dma_start(out=smallrt[0:1, C:C + C_CTX], in_=g2r[:])
            vals_sb = small.tile([128, NCH], f32, tag="vals_sb")
            nc.sync.dma_start(
                out=vals_sb[:],
                in_=smallrt[0:1, 0:C].rearrange("o (c p) -> p (o c)", p=128))
            g2col = small.tile([128, NKH], f32, tag="g2col")
            nc.sync.dma_start(
                out=g2col[:],
                in_=smallrt[0:1, C:C + C_CTX].rearrange("o (h p) -> p (o h)",
                                                        p=128))
            m1F = small.tile([128, NCH, NKH * 128], bf16, tag="m1F")
            for ci in range(NCH):
                nc.vector.tensor_scalar_mul(m1F[:, ci, :], m1S[:, ci, :],
                                            vals_sb[:, ci:ci + 1])
            # den' and eps*den'^2
            den = small.tile([1, SEC], f32, tag="den")
            prodq = small.tile([1, SEC, V], f32, tag="prodq")
            nc.vector.tensor_tensor(
                out=prodq[:], in0=cntS[:],
                in1=wvpr[:].unsqueeze(1).broadcast_to([1, SEC, V]),
                op=ALU.mult)
            nc.vector.tensor_reduce(out=den[:], in_=prodq[:],
                                    axis=mybir.AxisListType.X, op=ALU.add)
            nc.vector.tensor_scalar_add(den[:], den[:], FUSE_EPS)
            ed2 = small.tile([1, SEC], f32, tag="ed2")
            nc.vector.tensor_tensor(out=ed2[:], in0=den[:], in1=den[:],
                                    op=ALU.mult)
            nc.vector.tensor_scalar_mul(ed2[:], ed2[:], LN_EPS)

            # ---------------- acc -> SBUF (bf16), squares, K row
            accS = big.tile([128, NCH, SEC], bf16, tag="accS")
            nc.vector.tensor_copy(out=accS[:], in_=accP[:, :, 0:SEC])
            sqb = big.tile([128, NCH, SEC], bf16, tag="sqb")
            nc.scalar.activation(out=sqb[:], in_=accP[:, :, 0:SEC], func=AF.Square)

            # K = Sum_c acc / C (exact: num = acc - K has zero channel mean)
            # and Sum_c acc^2, both via ones-matmul rows.
            ssqP = psum.tile([128, 512], f32, tag="kbp")
            for ci in range(NCH):
                nc.tensor.matmul(miscP[0:1, 0:SEC],
                                 lhsT=onesS[:, 0:1], rhs=accS[:, ci, :],
                                 start=(ci == 0), stop=(ci == NCH - 1),
                                 skip_group_check=True)
            for ci in range(NCH):
                nc.tensor.matmul(ssqP[0:1, 0:SEC],
                                 lhsT=onesS[:, 0:1], rhs=sqb[:, ci, :],
                                 start=(ci == 0), stop=(ci == NCH - 1),
                                 skip_group_check=True)
            kr = small.tile([1, SEC], f32, tag="kr")
            nc.vector.tensor_scalar_mul(kr[:], miscP[0:1, 0:SEC], 1.0 / C)
            krb = small.tile([1, SEC], bf16, tag="krb")
            nc.vector.tensor_copy(out=krb[:], in_=kr[:])

            # ---------------- grouped reducer matmuls (+ K correction row)
            yP = psum.tile([128, NKH, 512], f32, tag="accp")
            for kh in range(NKH):
                nc.tensor.matmul(yP[:, kh, 0:SEC],
                                 lhsT=ng1b[0:1, 128 * kh:128 * (kh + 1)],
                                 rhs=krb[:],
                                 start=True, stop=False, skip_group_check=True)
                for ci in range(NCH):
                    nc.tensor.matmul(yP[:, kh, 0:SEC],
                                     lhsT=m1F[:, ci, 128 * kh:128 * (kh + 1)],
                                     rhs=accS[:, ci, :],
                                     start=False, stop=(ci == NCH - 1),
                                     skip_group_check=True)

            # ---------------- A = rsqrt(SSacc/C - K^2 + eps*den'^2)
            zq = small.tile([1, SEC], f32, tag="zq")
            u1 = small.tile([1, SEC], f32, tag="u1")
            nc.vector.tensor_tensor(out=u1[:], in0=kr[:], in1=kr[:], op=ALU.mult)
            nc.vector.tensor_scalar_mul(zq[:], ssqP[0:1, 0:SEC], 1.0 / C)
            nc.vector.tensor_tensor(out=zq[:], in0=zq[:], in1=u1[:],
                                    op=ALU.subtract)
            nc.vector.tensor_tensor(out=zq[:], in0=zq[:], in1=ed2[:], op=ALU.add)
            nc.scalar.activation(out=zq[:], in_=zq[:], func=AF.Ln)
            nc.scalar.activation(out=zq[:], in_=zq[:], func=AF.Exp, scale=-0.5)
            aQb = small.tile([1, SEC], bf16, tag="aQb")
            nc.vector.tensor_copy(out=aQb[:], in_=zq[:])
            abP = psum.tile([128, 512], f32, tag="kbp")
            nc.tensor.matmul(abP[:, 0:SEC], lhsT=onesrow[:], rhs=aQb[:],
                             start=True, stop=True, skip_group_check=True)
            abS = small.tile([128, SEC], f32, tag="abS")
            nc.scalar.copy(out=abS[:], in_=abP[:, 0:SEC])

            # ---------------- final scale/shift and output
            ySB = small.tile([128, NKH, SEC], f32, tag="ySB")
            nc.vector.tensor_tensor(
                out=ySB[:], in0=yP[:, :, 0:SEC],
                in1=abS[:].unsqueeze(1).broadcast_to([128, NKH, SEC]),
                op=ALU.mult)
            for kh in range(NKH):
                nc.vector.tensor_scalar_add(ySB[:, kh, :], ySB[:, kh, :],
                                            g2col[:, kh:kh + 1])
            out_v = out_d.ap().rearrange("p (h q) -> p h q", h=NKH)
            for kh in range(NKH):
                nc.sync.dma_start(out=out_v[:, kh, :], in_=ySB[:, kh, :])

    nc.compile()
    return nc





# ------------------------------------------------------------------- driver
def make_in_maps(inputs, plan):
    lt = np.asarray(inputs["last_tokens"], np.float32)
    gamma = np.asarray(inputs["post_gamma"], np.float32).ravel()
    beta = np.asarray(inputs["post_beta"], np.float32).ravel()
    logits = np.asarray(inputs["logits"], np.float32)
    w_view = np.asarray(inputs["w_view"], np.float32).ravel()

    NTIL = plan["NTIL"]
    toks = retile_tokens_sector(lt, plan)

    rowc = np.zeros((1, 3 * C + 8), np.float32)
    rowc[0, 0:C] = gamma
    rowc[0, C:2 * C] = beta
    rowc[0, 2 * C:3 * C] = logits.reshape(-1)
    rowc[0, 3 * C:3 * C + V] = w_view

    # m1mask[p, ci, kh*128+j] = 1 iff (128*kh + j) == (128*ci + p)//3
    cg = np.arange(C)
    m1mask = np.zeros((128, NCH, NKH * 128), ml_dtypes.bfloat16)
    for ci in range(NCH):
        p = np.arange(128)
        kg = (128 * ci + p) // 3
        kh = kg // 128
        j = kg % 128
        m1mask[p, ci, kh * 128 + j] = 1.0
    m1mask = m1mask.reshape(128, NCH * NKH * 128)

    onescol = np.ones((128, 8), ml_dtypes.bfloat16)

    in_maps = []
    for k in range(NCORE):
        ck = plan["cores"][k]
        wmat = ck["wmat"].transpose(1, 0, 2).reshape(128, NTIL * SEC)
        in_maps.append({
            "tok": toks[k].reshape(128, NTIL * C),
            "wmat": np.ascontiguousarray(wmat.astype(ml_dtypes.bfloat16)),
            "cnt": np.ascontiguousarray(
                ck["cntq"].reshape(1, SEC * V), np.float32),
            "rowc": rowc,
            "m1mask": np.ascontiguousarray(m1mask),
            "vmap": np.ascontiguousarray(
                np.broadcast_to(ck["vmap"].reshape(1, NTIL * V),
                                (128, NTIL * V)), np.float32),
            "onescol": onescol,
        })
    return in_maps


def assemble_output(results, plan):
    Y = np.zeros((Q, C_CTX), np.float32)
    perm = plan["perm"]
    for k in range(NCORE):
        arr = np.asarray(results[k]["out"], np.float32).reshape(128, NKH, SEC)
        qs = perm[k * SEC:min((k + 1) * SEC, Q)]
        nq = len(qs)
        # y[q, kh*128+p] = arr[p, kh, j]
        Y[qs] = arr[:, :, :nq].transpose(1, 0, 2).reshape(C_CTX, nq).T
    return np.ascontiguousarray(
        Y.reshape(1, BEV_H, BEV_W, C_CTX).transpose(0, 3, 1, 2))


_CACHE = {}


def _get_program(lidar2img, patch_h, patch_w):
    key = (lidar2img.tobytes(), int(patch_h), int(patch_w))
    if key not in _CACHE:
        plan = build_plan(lidar2img, patch_h, patch_w)
        nc = build_program(plan["NTIL"])
        _CACHE[key] = (plan, nc)
    return _CACHE[key]


def _install_ntff_shim():
    """Provide antenv.axon_hooks (absent in this image) so trace=True can
    capture NTFF profiles via the axon PJRT .so. Used only by test.py."""
    import types
    import ctypes
    import contextlib
    if "antenv.axon_hooks" in sys.modules:
        return
    so_path = "/opt/axon/libaxon_pjrt.so"
    lib = ctypes.CDLL(so_path)
    if not hasattr(lib, "axon_start_nrt_profile"):
        return
    lib.axon_start_nrt_profile.argtypes = [
        ctypes.POINTER(ctypes.c_int64), ctypes.c_size_t]
    lib.axon_start_nrt_profile.restype = ctypes.c_int64
    lib.axon_stop_nrt_profile.argtypes = [ctypes.c_char_p]
    lib.axon_stop_nrt_profile.restype = ctypes.c_int64

    @contextlib.contextmanager
    def _hook(output_dir, device_ids):
        import jax
        jax.devices()
        if device_ids:
            ids = (ctypes.c_int64 * len(device_ids))(*device_ids)
            rc = lib.axon_start_nrt_profile(ids, len(device_ids))
        else:
            rc = lib.axon_start_nrt_profile(None, 0)
        if rc != 0:
            raise RuntimeError(f"axon_start_nrt_profile rc={rc}")
        try:
            yield
        finally:
            n = lib.axon_stop_nrt_profile(str(output_dir).encode())
            print(f"ntff profile: {n} file(s) -> {output_dir}", file=sys.stderr)

    mod = types.ModuleType("antenv.axon_hooks")
    mod.get_axon_ntff_profile_hook = lambda: _hook
    mod.set_axon_ntff_profile_hook = lambda h: None
    sys.modules["antenv.axon_hooks"] = mod
    import antenv
    antenv.axon_hooks = mod


def kernel(last_tokens, lidar2img, w_view, post_gamma, post_beta, logits,
           patch_h, patch_w, _trace=False):
    import concourse.bass_utils as bu
    from concourse.bass_utils import run_bass_kernel_spmd
    if _trace:
        _install_ntff_shim()
        bu.upload_artifacts = lambda tmpdir: "local://" + str(tmpdir)
    inputs = dict(last_tokens=np.asarray(last_tokens),
                  lidar2img=np.asarray(lidar2img, np.float32),
                  w_view=w_view, post_gamma=post_gamma, post_beta=post_beta,
                  logits=logits, patch_h=patch_h, patch_w=patch_w)
    plan, nc = _get_program(inputs["lidar2img"], patch_h, patch_w)
    in_maps = make_in_maps(inputs, plan)
    res = run_bass_kernel_spmd(nc, in_maps, core_ids=list(range(NCORE)),
                               trace=_trace)
    out = assemble_output(res.results, plan)
    kernel.last_result = res
    return out

